# revision 20
# baseline (speedup 1.0000x reference)
"""Block-sparse matmul + bias + relu on 8 Trainium2 NeuronCores.

Strategy (data-parallel over batch, wave-synchronous PE schedule):
  - Shard x along batch: 8 cores x 512 rows. w_blocks/bias replicated.
  - Per core, out^T computed with the PE in 32x32 tiling mode. The PE's
    128x128 array is 16 independent 32x32 subarrays; a full-array
    LDWEIGHTS [128,128] loads 16 blocks' weights at once (~107ns, FWL),
    vs ~27ns each for 16 separate 32-col loads that serialize against
    in-flight matmuls on the same row group.
  - Waves: each wave = 1 standalone LDWEIGHTS (16 weight blocks packed
    in the wim image) + up to 16 MATMULs (one per 32x32 tile position),
    marked ldweights=False so they use the batched load. All 16 run
    concurrently (span ~ one MM dur + issue stagger). The next wave's
    LDWEIGHTS can't be pulled ahead by HW (full-array row-group
    conflict), giving a natural wave barrier; explicit NO_SYNC deps pin
    the Tile scheduler to the same order.
  - PSUM: each output block-col j is hosted on one (bank, strip) slot
    for its lifetime; block (i,j) runs at tile (i%4, strip(j)) and
    accumulates into that slot. Cross-row-group writes to the same slot
    are serialized by the wave barrier, so no cross-bank combine is
    needed: the epilogue is ONE scalar-engine activation (relu(acc +
    bias), PSUM->SBUF bf16) per bank 'generation' (the <=4 cols
    concurrently hosted on its strips, formed size-matched so they
    drain together; refill waits GAP waves after the ACT). 32 slots in
    flight; a greedy max-matching packs each wave with <=1 block per
    tile and <=1 block per hosted col.
  - Per-matmul semaphore increments are stripped post-Tile (only
    increments some wait references are kept; matmuls complete in pc
    order so rank-remapped thresholds are equivalent).
  - Stores batched 2 quads per DMA descriptor in quad-completion order
    (host reorders); inputs double-buffered (const pool bufs=2) so the
    next sweep's x/w loads overlap this sweep's compute.
  - Output returned as bf16 from device, upcast to fp32 on host.
"""

import os
from collections import defaultdict, deque

import numpy as np
import ml_dtypes

import concourse.bass as bass
import concourse.tile as tile
from concourse import mybir
from concourse.bass_utils import run_bass_kernel_spmd

LAST_RESULTS = None  # test-only: BassKernelResults of the last run

BS = 32
KB = 128
NB = 128
BATCH = 4096
NCORES = 8
BC = BATCH // NCORES          # 512 batch rows per core
NQ = NB // 4                  # 32 quads of output block-cols
NBANKS = 8
IN_DT = mybir.dt.bfloat16
IN_NP = ml_dtypes.bfloat16
OUT_DT = mybir.dt.bfloat16
OUT_NP = ml_dtypes.bfloat16

_CACHE = {}


GAP = 2  # waves between a bank's ACT and its slots' reuse (hides ACT+sem)


def _build_schedule(row_idx, col_idx):
    """Wave-packing over 32 (bank, strip) PSUM slots. Each output col j is
    hosted by one slot for its lifetime; a bank's 'generation' is the set of
    <=4 cols concurrently hosted on its 4 strips, drained by ONE activation
    (relu(acc+bias)) reading the whole bank. Generations are formed from the
    largest pending cols (size-matched so they finish together) and refill
    GAP waves after the ACT so the ACT latency never stalls the PE queue.
    Each wave picks at most one block per 32x32 tile position (r=i%4, free
    strip), and at most one block per hosted col: per strip c, a max
    bipartite matching assigns the 4 row cells to distinct cols hosted at
    strip c.

    Returns (sched, S, slot_of, dummy_slots):
      sched = {'waves': [[(r, c, bank, t, start, stop), ...]],
               'acts': [(wave, bank, {strip: col}), ...]};  S = #waves;
      slot_of[n] = (wave, r, c) weight-image slot of block n;
      dummy_slots kept for interface compat (always empty)."""
    nnz = len(row_idx)
    col_cells = [[deque() for _ in range(4)] for _ in range(NB)]  # [j][r]
    col_rem = [0] * NB
    for n in range(nnz):
        i = int(row_idx[n]); j = int(col_idx[n])
        col_cells[j][i % 4].append((n, i // 4))
        col_rem[j] += 1
    for j in range(NB):
        if col_rem[j] == 0:
            # empty output col: one zero-weight matmul defines its PSUM slot
            col_cells[j][0].append((None, 0))
            col_rem[j] = 1

    pending = sorted(range(NB), key=lambda j: -col_rem[j])
    hosted = [[None] * 4 for _ in range(NBANKS)]   # [b][c] -> col or None
    gen_cols = [dict() for _ in range(NBANKS)]     # strip -> col (this gen)
    gen_rem = [0] * NBANKS                          # unfinished cols in gen
    gen_of = {}                                     # col -> bank
    refill_at = [0] * NBANKS
    col_started = [False] * NB
    waves, acts = [], []
    slot_of, dummy_slots = {}, []

    def form_generation(b, w):
        take = pending[:4]
        del pending[:4]
        # largest col onto the strip with the least outstanding work
        loads = []
        for c in range(4):
            tot = sum(col_rem[hosted[bb][c]] for bb in range(NBANKS)
                      if hosted[bb][c] is not None)
            loads.append((tot, c))
        loads.sort()
        gen_cols[b] = {}
        for (ld, c), j in zip(loads, sorted(take, key=lambda j: -col_rem[j])):
            hosted[b][c] = j
            gen_cols[b][c] = j
            gen_of[j] = b
        gen_rem[b] = len(take)

    for b in range(NBANKS):
        form_generation(b, 0)

    while any(gen_rem) or pending:
        w = len(waves)
        for b in range(NBANKS):
            if gen_rem[b] == 0 and pending and refill_at[b] <= w:
                form_generation(b, w)
        wave = []
        for c in range(4):
            # candidate cols per row cell: prefer the critical col of each
            # generation (the one gating its bank's ACT), then generations
            # closest to done, then longer cell queues
            cands = {}
            for r in range(4):
                lst = [hosted[b][c] for b in range(NBANKS)
                       if hosted[b][c] is not None
                       and col_cells[hosted[b][c]][r]]

                def pref(j):
                    b = gen_of[j]
                    live = [col_rem[jj] for jj in gen_cols[b].values()
                            if col_rem[jj] > 0]
                    is_crit = 1 if col_rem[j] == max(live) else 0
                    return (-is_crit, sum(live), -len(col_cells[j][r]))

                lst.sort(key=pref)
                cands[r] = lst
            matchj = {}  # col -> row

            def try_assign(r, visited):
                for j in cands[r]:
                    if j in visited:
                        continue
                    visited.add(j)
                    if j not in matchj or try_assign(matchj[j], visited):
                        matchj[j] = r
                        return True
                return False

            for r in sorted(range(4), key=lambda r: len(cands[r])):
                if cands[r]:
                    try_assign(r, set())
            for j, r in matchj.items():
                b = next(bb for bb in range(NBANKS) if hosted[bb][c] == j)
                n, t = col_cells[j][r].popleft()
                start = not col_started[j]
                col_started[j] = True
                col_rem[j] -= 1
                stop = col_rem[j] == 0
                if n is not None:
                    slot_of[n] = (w, r, c)
                wave.append((r, c, b, t, start, stop))
                if stop:
                    hosted[b][c] = None
                    gen_rem[b] -= 1
                    if gen_rem[b] == 0:
                        acts.append((w, b, dict(gen_cols[b])))
                        refill_at[b] = w + 1 + GAP
        waves.append(wave)

    # static race check: unique tile and unique (bank, strip) per wave
    for wave in waves:
        seen_rc, seen_bc = set(), set()
        for (r, c, b, t, start, stop) in wave:
            assert (r, c) not in seen_rc and (b, c) not in seen_bc
            seen_rc.add((r, c)); seen_bc.add((b, c))
    sched = {"waves": waves, "acts": acts}
    # acts doubles as the aux "dummy_slots" return so callers can thread it
    # to _prep_inputs unchanged (bias image needs the activation schedule).
    return sched, len(waves), slot_of, acts


_MULTIWAIT_OK = {"InstDMACopy", "InstUnconditionalBranch",
                 "InstConditionalBranch"}


def _legalize_waits(nc):
    """Engine ISA structs carry a single sync-wait slot; Tile can emit more.
    Offload excess waits onto same-engine NoOps inserted just before the
    instruction (per-engine stream order is the block list order)."""
    ctr = 0
    for f in nc.m.functions:
        for blk in f.blocks:
            out = []
            for inst in blk.instructions:
                si = inst.sync_info
                if (si is not None and si.on_wait and len(si.on_wait) > 1
                        and type(inst).__name__ == "InstDMACopy"):
                    # HWDGE lane sems are monotonic add-only counters; a
                    # DMA's wait on its own completion lane orders it against
                    # unrelated prior DMAs on that lane and is droppable.
                    own = {u.ant_name for u in (si.on_update or [])}
                    keep = [w for w in si.on_wait if w.ant_name not in own]
                    if len(keep) > 1:
                        raise RuntimeError(
                            f"DMA {inst.name} still has waits {keep}")
                    inst.sync_info = mybir.SyncInfo(on_wait=keep,
                                                    on_update=si.on_update)
                    out.append(inst)
                    continue
                if (si is not None and si.on_wait and len(si.on_wait) > 1
                        and type(inst).__name__ not in _MULTIWAIT_OK):
                    waits = list(si.on_wait)
                    for w in waits[:-1]:
                        nop = mybir.InstNoOp(name=f"waitnop-{ctr}")
                        ctr += 1
                        nop.engine = inst.engine
                        nop.sync_info = mybir.SyncInfo(on_wait=[w], on_update=[])
                        out.append(nop)
                    inst.sync_info = mybir.SyncInfo(on_wait=[waits[-1]],
                                                    on_update=si.on_update)
                out.append(inst)
            blk.instructions[:] = out


_PC_ORDERED_PE = {"InstMatmult", "InstLdweights"}


def _strip_mm_sem_incs(nc):
    """Drop per-matmul/ldweights semaphore increments except those whose
    count some wait references; remap wait thresholds and the loop
    pre-credit/debit amounts to the compacted count. Matmuls complete in pc
    order, and a full-array LDWEIGHTS starts only after all prior matmuls
    drain (row-group conflict) and finishes before its own wave's matmuls,
    so PE-stream completions are pc-ordered and waiting for the rank(v)-th
    kept increment is equivalent to the original v-th."""
    mm_sems = set()
    for f in nc.m.functions:
        for blk in f.blocks:
            for inst in blk.instructions:
                if (getattr(inst, 'engine', None) == mybir.EngineType.PE
                        and type(inst).__name__ in _PC_ORDERED_PE):
                    si = inst.sync_info
                    if si is None:
                        continue
                    for u in (si.on_update or []):
                        if u.update_mode == "sem-inc" and u.update_value == 1:
                            mm_sems.add(u.ant_name)
    for sem in mm_sems:
        ok = True
        total = 0
        thresholds = set()
        adjusts = []
        mm_counts = {}
        for f in nc.m.functions:
            for blk in f.blocks:
                c = 0
                for inst in blk.instructions:
                    si = inst.sync_info
                    if si is None:
                        continue
                    for u in (si.on_update or []):
                        if u.ant_name != sem:
                            continue
                        if u.update_mode == "sem-inc" and u.update_value == 1:
                            if type(inst).__name__ not in _PC_ORDERED_PE:
                                ok = False
                            c += 1
                            mm_counts[id(inst)] = c
                        elif u.update_mode in ("sem-add-imm", "sem-sub-imm"):
                            adjusts.append(u)
                        else:
                            ok = False
                    for w in (si.on_wait or []):
                        if w.ant_name == sem and w.wait_mode != "sem-ge-imm":
                            ok = False
                total = max(total, c)
        for f in nc.m.functions:
            for blk in f.blocks:
                for inst in blk.instructions:
                    si = inst.sync_info
                    if si is None:
                        continue
                    for w in (si.on_wait or []):
                        if w.ant_name == sem and w.wait_value > 0:
                            thresholds.add(w.wait_value)
        if not ok or total == 0:
            continue
        if any(u.update_value != total for u in adjusts):
            continue
        keep = sorted(thresholds | {total})
        rank = {v: i + 1 for i, v in enumerate(keep)}
        newtotal = len(keep)
        for f in nc.m.functions:
            for blk in f.blocks:
                for inst in blk.instructions:
                    si = inst.sync_info
                    if si is None:
                        continue
                    changed = False
                    new_waits = []
                    for w in (si.on_wait or []):
                        if w.ant_name == sem and w.wait_value > 0:
                            new_waits.append(mybir.SyncWait(
                                sync_type=w.sync_type, id=w.id,
                                ant_name=w.ant_name, wait_mode=w.wait_mode,
                                wait_value=rank[w.wait_value],
                                wait_reg=w.wait_reg))
                            changed = True
                        else:
                            new_waits.append(w)
                    new_upds = []
                    for u in (si.on_update or []):
                        if u.ant_name != sem:
                            new_upds.append(u)
                            continue
                        if u.update_mode == "sem-inc":
                            if mm_counts[id(inst)] in rank:
                                new_upds.append(u)
                            changed = True
                        elif u.update_mode in ("sem-add-imm", "sem-sub-imm"):
                            new_upds.append(mybir.SyncUpdate(
                                sync_type=u.sync_type, id=u.id,
                                ant_name=u.ant_name,
                                update_mode=u.update_mode,
                                update_value=newtotal,
                                update_reg=u.update_reg))
                            changed = True
                        else:
                            new_upds.append(u)
                    if changed:
                        inst.sync_info = mybir.SyncInfo(on_wait=new_waits,
                                                        on_update=new_upds)
    return nc


def _strip_auto_ldws(nc, keep_names):
    """tile_legalize splits every InstMatmult into a per-MM InstLdweights
    (32x32 tile load) + InstMatmult(ldweights=False). The per-MM loads are
    redundant with the batched full-array LDWEIGHTS emitted per wave (which
    loads identical weights for all 16 tiles in one 128-col stream), so drop
    them, merging any sync waits/updates onto the paired matmul. The batched
    load is always the first reader of its wt region, so DMA-arrival waits
    already land on it, not on the per-MM loads."""
    for f in nc.m.functions:
        for blk in f.blocks:
            out = []
            pending = []
            for inst in blk.instructions:
                if (type(inst).__name__ == "InstLdweights"
                        and inst.name not in keep_names):
                    si = inst.sync_info
                    if si is not None and (si.on_wait or si.on_update):
                        pending.append(si)
                    continue
                if pending:
                    si = inst.sync_info
                    waits = list(si.on_wait) if si else []
                    upds = list(si.on_update) if si else []
                    for ps in pending:
                        waits = list(ps.on_wait) + waits
                        upds = list(ps.on_update) + upds
                    inst.sync_info = mybir.SyncInfo(on_wait=waits,
                                                    on_update=upds)
                    pending = []
                out.append(inst)
            assert not pending, "dangling sync from stripped LDWEIGHTS"
            blk.instructions[:] = out


def _verify_pe_order(nc, wave_names):
    """Assert the post-Tile PE stream preserves the wave structure: every
    wave's LDWEIGHTS comes after all of the previous wave's matmuls and
    before all of its own (HW executes the PE queue in order; an
    out-of-order LDWEIGHTS would clobber weights of in-flight matmuls)."""
    pos = {}
    k = 0
    for f in nc.m.functions:
        for blk in f.blocks:
            for inst in blk.instructions:
                if getattr(inst, 'engine', None) == mybir.EngineType.PE:
                    pos[inst.name] = k
                    k += 1
    prev_mms = []
    for (ldw_name, mm_names) in wave_names:
        if ldw_name not in pos:
            raise RuntimeError(f"missing PE inst {ldw_name}")
        p_ldw = pos[ldw_name]
        for m in prev_mms:
            if pos[m] >= p_ldw:
                raise RuntimeError(
                    f"PE order violation: {m} scheduled after {ldw_name}")
        for m in mm_names:
            if pos[m] <= p_ldw:
                raise RuntimeError(
                    f"PE order violation: {m} scheduled before {ldw_name}")
        prev_mms = mm_names


def _build_program(sched, S, repeat=1, loop_n=0):
    import contextlib
    waves = sched["waves"]
    acts = sched["acts"]
    acts_after = defaultdict(list)
    for rank, (w, b, colmap) in enumerate(acts):
        acts_after[w].append((b, rank))
    n_total_acts = len(acts)
    NA = n_total_acts

    nc = bass.Bass("TRN2", target_bir_lowering=False, debug=False,
                   num_devices=NCORES)
    x_d = nc.dram_tensor("xt", [128, 32 * BC], IN_DT, kind="ExternalInput").ap()
    w_d = nc.dram_tensor("wim", [128, S * 128], IN_DT, kind="ExternalInput").ap()
    b_d = nc.dram_tensor("bias", [128, NA], mybir.dt.float32,
                         kind="ExternalInput").ap()
    o_d = nc.dram_tensor("outT", [128, NA * BC], OUT_DT, kind="ExternalOutput").ap()

    NOSYNC = mybir.DependencyInfo.NO_SYNC_ONLY
    all_wave_names = []
    with tile.TileContext(nc) as tc:
        loop_cm = tc.For_i(0, loop_n, 1) if loop_n else contextlib.nullcontext()
        with tc.tile_pool(name="const", bufs=2) as cpool, \
             tc.tile_pool(name="work", bufs=4) as wpool, \
             tc.tile_pool(name="psum", bufs=1, space="PSUM") as ppool, \
             loop_cm:
            xt = cpool.tile([128, 32 * BC], IN_DT)
            wt = cpool.tile([128, S * 128], IN_DT)
            bt = cpool.tile([128, NA], mybir.dt.float32)
            nc.sync.dma_start(bt[:], b_d[:])
            # x: chunked DMA (8 x 1MB)
            xch = (32 * BC) // 8
            for k in range(8):
                nc.sync.dma_start(xt[:, k * xch:(k + 1) * xch],
                                  x_d[:, k * xch:(k + 1) * xch])
            # w: chunked DMA in wave order so early waves unblock early,
            # on the Activation HWDGE queue so x and w transfers use both
            # DMA queues in parallel
            wstep = -(-S // 8) * 128
            for k in range(8):
                lo = k * wstep
                hi = min(S * 128, lo + wstep)
                if lo >= hi:
                    continue
                nc.scalar.dma_start(wt[:, lo:hi], w_d[:, lo:hi])

            for rep in range(repeat):
                cur = {b: None for b in range(NBANKS)}
                gen = [0] * NBANKS
                prev_mms = []
                act_rank = 0
                otb = None
                for w, wave in enumerate(waves):
                    ldw = nc.tensor.ldweights(wt[:, 128 * w:128 * (w + 1)],
                                              tile_position=(0, 0))
                    ldw.ins.tile_size = (128, 128)
                    for m in prev_mms:
                        ldw.ins.add_dependency(m, NOSYNC)
                    mm_names = []
                    for (r, c, b, t, start, stop) in wave:
                        if cur[b] is None:
                            cur[b] = ppool.tile(
                                [128, BC], mybir.dt.float32, tag=f"acc{b}",
                                name=f"acc{b}_g{gen[b]}_p{rep}")
                            gen[b] += 1
                        mm = nc.tensor.matmul(
                            out=cur[b][32 * c:32 * c + 32, :],
                            lhsT=wt[32 * r:32 * r + 32,
                                    128 * w + 32 * c:128 * w + 32 * c + 32],
                            rhs=xt[32 * r:32 * r + 32, t * BC:(t + 1) * BC],
                            start=start, stop=stop,
                            tile_position=(32 * r, 32 * c),
                            skip_group_check=True,
                        )
                        mm.ins.ldweights = False
                        mm.ins.add_dependency(ldw.ins.name, NOSYNC)
                        mm_names.append(mm.ins.name)
                    all_wave_names.append((ldw.ins.name, mm_names))
                    prev_mms = mm_names
                    for (b, rank) in acts_after.get(w, []):
                        if act_rank % 2 == 0:
                            otb = wpool.tile([128, 2 * BC], OUT_DT, tag="otb")
                        ot = otb[:, (act_rank % 2) * BC:(act_rank % 2 + 1) * BC]
                        nc.scalar.activation(ot, cur[b][:],
                                             mybir.ActivationFunctionType.Relu,
                                             bias=bt[:, rank:rank + 1], scale=1.0)
                        if act_rank % 2 == 1:
                            # store issued from the ACT engine: same-engine
                            # dependency on the producer, no cross-engine sem
                            nc.scalar.dma_start(
                                o_d[:, (act_rank - 1) * BC:(act_rank + 1) * BC],
                                otb[:])
                        elif act_rank == n_total_acts - 1:
                            nc.scalar.dma_start(
                                o_d[:, act_rank * BC:(act_rank + 1) * BC],
                                otb[:, 0:BC])
                        act_rank += 1
                        cur[b] = None
                assert act_rank == n_total_acts
    _verify_pe_order(nc, all_wave_names)
    keep = {ldw for (ldw, mms) in all_wave_names}
    _strip_auto_ldws(nc, keep)
    _legalize_waits(nc)
    _strip_mm_sem_incs(nc)
    return nc


def _prep_inputs(x, w_blocks, bias, row_idx, col_idx, slot_of, dummy_slots, S):
    nnz = len(row_idx)
    acts = dummy_slots  # aux slot reused to carry the activation schedule
    # x^T images per core: block i at partitions 32*(i%4), free tile i//4.
    xb = x.astype(IN_NP).reshape(BATCH, 32, 4, 32)        # b, t, r, p
    xt_all = np.ascontiguousarray(xb.transpose(2, 3, 1, 0))  # r, p, t, b
    xt_all = xt_all.reshape(128, 32, BATCH)
    xts = [np.ascontiguousarray(xt_all[:, :, c * BC:(c + 1) * BC]
                                ).reshape(128, 32 * BC) for c in range(NCORES)]
    # w image [128, S*128]: wave w, tile (r, c) at [32r:, 128w+32c:]
    wim = np.zeros((128, S * 128), dtype=IN_NP)
    wb = w_blocks.astype(IN_NP)
    for n in range(nnz):
        w, r, c = slot_of[n]
        wim[32 * r:32 * r + 32, 128 * w + 32 * c:128 * w + 32 * c + 32] = wb[n]
    # bias image [128, n_acts]: ACT rank g hosts cols colmap[c] at strips c
    bias = bias.astype(np.float32)
    bim = np.zeros((128, len(acts)), dtype=np.float32)
    for rank, (w, b, colmap) in enumerate(acts):
        for c, j in colmap.items():
            bim[32 * c:32 * c + 32, rank] = bias[32 * j:32 * j + 32]
    return xts, wim, bim


def kernel(x, w_blocks, bias, row_idx, col_idx):
    x = np.asarray(x)
    w_blocks = np.asarray(w_blocks)
    bias = np.asarray(bias)
    row_idx = np.asarray(row_idx)
    col_idx = np.asarray(col_idx)
    key = (row_idx.tobytes(), col_idx.tobytes())
    if key not in _CACHE:
        sched, S, slot_of, dummy_slots = _build_schedule(row_idx, col_idx)
        nc = _build_program(sched, S)
        _CACHE[key] = (nc, S, (sched, slot_of, dummy_slots))
    nc, S, aux = _CACHE[key]
    sched, slot_of, dummy_slots = aux
    acts = sched["acts"]

    xts, wim, bim = _prep_inputs(x, w_blocks, bias, row_idx, col_idx,
                                 slot_of, acts, S)
    in_maps = [{"xt": xts[c], "wim": wim, "bias": bim} for c in range(NCORES)]
    res = run_bass_kernel_spmd(nc, in_maps, list(range(NCORES)))
    global LAST_RESULTS
    LAST_RESULTS = res

    out = np.empty((BATCH, NB * BS), dtype=np.float32)
    for c in range(NCORES):
        arr = res.results[c]["outT"].reshape(128, len(acts), BC)
        oc = out[c * BC:(c + 1) * BC, :]
        for rank, (w, b, colmap) in enumerate(acts):
            for st, j in colmap.items():
                oc[:, 32 * j:32 * j + 32] = (
                    arr[32 * st:32 * st + 32, rank, :].T.astype(np.float32))
    return out


# revision 22
# speedup vs baseline: 1.0660x; 1.0660x over previous
"""Block-sparse matmul + bias + relu on 8 Trainium2 NeuronCores.

Strategy (data-parallel over batch, wave-synchronous PE schedule):
  - Shard x along batch: 8 cores x 512 rows. w_blocks/bias replicated.
  - Per core, out^T computed with the PE in 32x32 tiling mode. The PE's
    128x128 array is 16 independent 32x32 subarrays; a full-array
    LDWEIGHTS [128,128] loads 16 blocks' weights at once (~107ns, FWL),
    vs ~27ns each for 16 separate 32-col loads that serialize against
    in-flight matmuls on the same row group.
  - Waves: each wave = 1 standalone LDWEIGHTS (16 weight blocks packed
    in the wim image) + up to 16 MATMULs (one per 32x32 tile position),
    marked ldweights=False so they use the batched load. All 16 run
    concurrently (span ~ one MM dur + issue stagger). The next wave's
    LDWEIGHTS can't be pulled ahead by HW (full-array row-group
    conflict), giving a natural wave barrier; explicit NO_SYNC deps pin
    the Tile scheduler to the same order.
  - PSUM: each output block-col j is hosted on one (bank, strip) slot
    for its lifetime; block (i,j) runs at tile (i%4, strip(j)) and
    accumulates into that slot. Cross-row-group writes to the same slot
    are serialized by the wave barrier, so no cross-bank combine is
    needed: the epilogue is ONE scalar-engine activation (relu(acc +
    bias), PSUM->SBUF bf16) per bank 'generation' (the <=4 cols
    concurrently hosted on its strips, formed size-matched so they
    drain together; refill waits GAP waves after the ACT). 32 slots in
    flight; a greedy max-matching packs each wave with <=1 block per
    tile and <=1 block per hosted col.
  - Per-matmul semaphore increments are stripped post-Tile (only
    increments some wait references are kept; matmuls complete in pc
    order so rank-remapped thresholds are equivalent).
  - Stores batched 2 quads per DMA descriptor in quad-completion order
    (host reorders); inputs double-buffered (const pool bufs=2) so the
    next sweep's x/w loads overlap this sweep's compute.
  - Output returned as bf16 from device, upcast to fp32 on host.
"""

import os
from collections import defaultdict, deque

import numpy as np
import ml_dtypes

import concourse.bass as bass
import concourse.tile as tile
from concourse import mybir
from concourse.bass_utils import run_bass_kernel_spmd

LAST_RESULTS = None  # test-only: BassKernelResults of the last run

BS = 32
KB = 128
NB = 128
BATCH = 4096
NCORES = 8
BC = BATCH // NCORES          # 512 batch rows per core
NQ = NB // 4                  # 32 quads of output block-cols
NBANKS = 8
IN_DT = mybir.dt.bfloat16
IN_NP = ml_dtypes.bfloat16
OUT_DT = mybir.dt.bfloat16
OUT_NP = ml_dtypes.bfloat16

_CACHE = {}


GAP = 2  # waves between a bank's ACT and its slots' reuse (hides ACT+sem)
DMA_SPLIT = False  # ACT-queue split measured +7us/sweep worse (paired A/B)


def _build_schedule(row_idx, col_idx):
    """Wave-packing over 32 (bank, strip) PSUM slots. Each output col j is
    hosted by one slot for its lifetime; a bank's 'generation' is the set of
    <=4 cols concurrently hosted on its 4 strips, drained by ONE activation
    (relu(acc+bias)) reading the whole bank. Generations are formed from the
    largest pending cols (size-matched so they finish together) and refill
    GAP waves after the ACT so the ACT latency never stalls the PE queue.
    Each wave picks at most one block per 32x32 tile position (r=i%4, free
    strip), and at most one block per hosted col: per strip c, a max
    bipartite matching assigns the 4 row cells to distinct cols hosted at
    strip c.

    Returns (sched, S, slot_of, dummy_slots):
      sched = {'waves': [[(r, c, bank, t, start, stop), ...]],
               'acts': [(wave, bank, {strip: col}), ...]};  S = #waves;
      slot_of[n] = (wave, r, c) weight-image slot of block n;
      dummy_slots kept for interface compat (always empty)."""
    nnz = len(row_idx)
    col_cells = [[deque() for _ in range(4)] for _ in range(NB)]  # [j][r]
    col_rem = [0] * NB
    for n in range(nnz):
        i = int(row_idx[n]); j = int(col_idx[n])
        col_cells[j][i % 4].append((n, i // 4))
        col_rem[j] += 1
    for j in range(NB):
        if col_rem[j] == 0:
            # empty output col: one zero-weight matmul defines its PSUM slot
            col_cells[j][0].append((None, 0))
            col_rem[j] = 1

    pending = sorted(range(NB), key=lambda j: -col_rem[j])
    hosted = [[None] * 4 for _ in range(NBANKS)]   # [b][c] -> col or None
    gen_cols = [dict() for _ in range(NBANKS)]     # strip -> col (this gen)
    gen_rem = [0] * NBANKS                          # unfinished cols in gen
    gen_of = {}                                     # col -> bank
    refill_at = [0] * NBANKS
    col_started = [False] * NB
    waves, acts = [], []
    slot_of, dummy_slots = {}, []

    def form_generation(b, w):
        take = pending[:4]
        del pending[:4]
        # largest col onto the strip with the least outstanding work
        loads = []
        for c in range(4):
            tot = sum(col_rem[hosted[bb][c]] for bb in range(NBANKS)
                      if hosted[bb][c] is not None)
            loads.append((tot, c))
        loads.sort()
        gen_cols[b] = {}
        for (ld, c), j in zip(loads, sorted(take, key=lambda j: -col_rem[j])):
            hosted[b][c] = j
            gen_cols[b][c] = j
            gen_of[j] = b
        gen_rem[b] = len(take)

    for b in range(NBANKS):
        form_generation(b, 0)

    while any(gen_rem) or pending:
        w = len(waves)
        for b in range(NBANKS):
            if gen_rem[b] == 0 and pending and refill_at[b] <= w:
                form_generation(b, w)
        wave = []
        for c in range(4):
            # candidate cols per row cell: prefer the critical col of each
            # generation (the one gating its bank's ACT), then generations
            # closest to done, then longer cell queues
            cands = {}
            for r in range(4):
                lst = [hosted[b][c] for b in range(NBANKS)
                       if hosted[b][c] is not None
                       and col_cells[hosted[b][c]][r]]

                def pref(j):
                    b = gen_of[j]
                    live = [col_rem[jj] for jj in gen_cols[b].values()
                            if col_rem[jj] > 0]
                    is_crit = 1 if col_rem[j] == max(live) else 0
                    return (-is_crit, sum(live), -len(col_cells[j][r]))

                lst.sort(key=pref)
                cands[r] = lst
            matchj = {}  # col -> row

            def try_assign(r, visited):
                for j in cands[r]:
                    if j in visited:
                        continue
                    visited.add(j)
                    if j not in matchj or try_assign(matchj[j], visited):
                        matchj[j] = r
                        return True
                return False

            for r in sorted(range(4), key=lambda r: len(cands[r])):
                if cands[r]:
                    try_assign(r, set())
            for j, r in matchj.items():
                b = next(bb for bb in range(NBANKS) if hosted[bb][c] == j)
                n, t = col_cells[j][r].popleft()
                start = not col_started[j]
                col_started[j] = True
                col_rem[j] -= 1
                stop = col_rem[j] == 0
                if n is not None:
                    slot_of[n] = (w, r, c)
                wave.append((r, c, b, t, start, stop))
                if stop:
                    hosted[b][c] = None
                    gen_rem[b] -= 1
                    if gen_rem[b] == 0:
                        acts.append((w, b, dict(gen_cols[b])))
                        refill_at[b] = w + 1 + GAP
        waves.append(wave)

    # static race check: unique tile and unique (bank, strip) per wave
    for wave in waves:
        seen_rc, seen_bc = set(), set()
        for (r, c, b, t, start, stop) in wave:
            assert (r, c) not in seen_rc and (b, c) not in seen_bc
            seen_rc.add((r, c)); seen_bc.add((b, c))
    sched = {"waves": waves, "acts": acts}
    # acts doubles as the aux "dummy_slots" return so callers can thread it
    # to _prep_inputs unchanged (bias image needs the activation schedule).
    return sched, len(waves), slot_of, acts


_MULTIWAIT_OK = {"InstDMACopy", "InstUnconditionalBranch",
                 "InstConditionalBranch"}


def _legalize_waits(nc):
    """Engine ISA structs carry a single sync-wait slot; Tile can emit more.
    Offload excess waits onto same-engine NoOps inserted just before the
    instruction (per-engine stream order is the block list order)."""
    ctr = 0
    for f in nc.m.functions:
        for blk in f.blocks:
            out = []
            for inst in blk.instructions:
                si = inst.sync_info
                if (si is not None and si.on_wait and len(si.on_wait) > 1
                        and type(inst).__name__ == "InstDMACopy"):
                    # HWDGE lane sems are monotonic add-only counters; a
                    # DMA's wait on its own completion lane orders it against
                    # unrelated prior DMAs on that lane and is droppable.
                    own = {u.ant_name for u in (si.on_update or [])}
                    keep = [w for w in si.on_wait if w.ant_name not in own]
                    if len(keep) > 1:
                        raise RuntimeError(
                            f"DMA {inst.name} still has waits {keep}")
                    inst.sync_info = mybir.SyncInfo(on_wait=keep,
                                                    on_update=si.on_update)
                    out.append(inst)
                    continue
                if (si is not None and si.on_wait and len(si.on_wait) > 1
                        and type(inst).__name__ not in _MULTIWAIT_OK):
                    waits = list(si.on_wait)
                    for w in waits[:-1]:
                        nop = mybir.InstNoOp(name=f"waitnop-{ctr}")
                        ctr += 1
                        nop.engine = inst.engine
                        nop.sync_info = mybir.SyncInfo(on_wait=[w], on_update=[])
                        out.append(nop)
                    inst.sync_info = mybir.SyncInfo(on_wait=[waits[-1]],
                                                    on_update=si.on_update)
                out.append(inst)
            blk.instructions[:] = out


_PC_ORDERED_PE = {"InstMatmult", "InstLdweights"}


def _strip_mm_sem_incs(nc):
    """Drop per-matmul/ldweights semaphore increments except those whose
    count some wait references; remap wait thresholds and the loop
    pre-credit/debit amounts to the compacted count. Matmuls complete in pc
    order, and a full-array LDWEIGHTS starts only after all prior matmuls
    drain (row-group conflict) and finishes before its own wave's matmuls,
    so PE-stream completions are pc-ordered and waiting for the rank(v)-th
    kept increment is equivalent to the original v-th."""
    mm_sems = set()
    for f in nc.m.functions:
        for blk in f.blocks:
            for inst in blk.instructions:
                if (getattr(inst, 'engine', None) == mybir.EngineType.PE
                        and type(inst).__name__ in _PC_ORDERED_PE):
                    si = inst.sync_info
                    if si is None:
                        continue
                    for u in (si.on_update or []):
                        if u.update_mode == "sem-inc" and u.update_value == 1:
                            mm_sems.add(u.ant_name)
    for sem in mm_sems:
        ok = True
        total = 0
        thresholds = set()
        adjusts = []
        mm_counts = {}
        for f in nc.m.functions:
            for blk in f.blocks:
                c = 0
                for inst in blk.instructions:
                    si = inst.sync_info
                    if si is None:
                        continue
                    for u in (si.on_update or []):
                        if u.ant_name != sem:
                            continue
                        if u.update_mode == "sem-inc" and u.update_value == 1:
                            if type(inst).__name__ not in _PC_ORDERED_PE:
                                ok = False
                            c += 1
                            mm_counts[id(inst)] = c
                        elif u.update_mode in ("sem-add-imm", "sem-sub-imm"):
                            adjusts.append(u)
                        else:
                            ok = False
                    for w in (si.on_wait or []):
                        if w.ant_name == sem and w.wait_mode != "sem-ge-imm":
                            ok = False
                total = max(total, c)
        for f in nc.m.functions:
            for blk in f.blocks:
                for inst in blk.instructions:
                    si = inst.sync_info
                    if si is None:
                        continue
                    for w in (si.on_wait or []):
                        if w.ant_name == sem and w.wait_value > 0:
                            thresholds.add(w.wait_value)
        if not ok or total == 0:
            continue
        if any(u.update_value != total for u in adjusts):
            continue
        keep = sorted(thresholds | {total})
        rank = {v: i + 1 for i, v in enumerate(keep)}
        newtotal = len(keep)
        for f in nc.m.functions:
            for blk in f.blocks:
                for inst in blk.instructions:
                    si = inst.sync_info
                    if si is None:
                        continue
                    changed = False
                    new_waits = []
                    for w in (si.on_wait or []):
                        if w.ant_name == sem and w.wait_value > 0:
                            new_waits.append(mybir.SyncWait(
                                sync_type=w.sync_type, id=w.id,
                                ant_name=w.ant_name, wait_mode=w.wait_mode,
                                wait_value=rank[w.wait_value],
                                wait_reg=w.wait_reg))
                            changed = True
                        else:
                            new_waits.append(w)
                    new_upds = []
                    for u in (si.on_update or []):
                        if u.ant_name != sem:
                            new_upds.append(u)
                            continue
                        if u.update_mode == "sem-inc":
                            if mm_counts[id(inst)] in rank:
                                new_upds.append(u)
                            changed = True
                        elif u.update_mode in ("sem-add-imm", "sem-sub-imm"):
                            new_upds.append(mybir.SyncUpdate(
                                sync_type=u.sync_type, id=u.id,
                                ant_name=u.ant_name,
                                update_mode=u.update_mode,
                                update_value=newtotal,
                                update_reg=u.update_reg))
                            changed = True
                        else:
                            new_upds.append(u)
                    if changed:
                        inst.sync_info = mybir.SyncInfo(on_wait=new_waits,
                                                        on_update=new_upds)
    return nc


def _strip_auto_ldws(nc, keep_names):
    """tile_legalize splits every InstMatmult into a per-MM InstLdweights
    (32x32 tile load) + InstMatmult(ldweights=False). The per-MM loads are
    redundant with the batched full-array LDWEIGHTS emitted per wave (which
    loads identical weights for all 16 tiles in one 128-col stream), so drop
    them, merging any sync waits/updates onto the paired matmul. The batched
    load is always the first reader of its wt region, so DMA-arrival waits
    already land on it, not on the per-MM loads."""
    for f in nc.m.functions:
        for blk in f.blocks:
            out = []
            pending = []
            for inst in blk.instructions:
                if (type(inst).__name__ == "InstLdweights"
                        and inst.name not in keep_names):
                    si = inst.sync_info
                    if si is not None and (si.on_wait or si.on_update):
                        pending.append(si)
                    continue
                if pending:
                    si = inst.sync_info
                    waits = list(si.on_wait) if si else []
                    upds = list(si.on_update) if si else []
                    for ps in pending:
                        waits = list(ps.on_wait) + waits
                        upds = list(ps.on_update) + upds
                    inst.sync_info = mybir.SyncInfo(on_wait=waits,
                                                    on_update=upds)
                    pending = []
                out.append(inst)
            assert not pending, "dangling sync from stripped LDWEIGHTS"
            blk.instructions[:] = out


def _verify_pe_order(nc, wave_names):
    """Assert the post-Tile PE stream preserves the wave structure: every
    wave's LDWEIGHTS comes after all of the previous wave's matmuls and
    before all of its own (HW executes the PE queue in order; an
    out-of-order LDWEIGHTS would clobber weights of in-flight matmuls)."""
    pos = {}
    k = 0
    for f in nc.m.functions:
        for blk in f.blocks:
            for inst in blk.instructions:
                if getattr(inst, 'engine', None) == mybir.EngineType.PE:
                    pos[inst.name] = k
                    k += 1
    prev_mms = []
    for (ldw_name, mm_names) in wave_names:
        if ldw_name not in pos:
            raise RuntimeError(f"missing PE inst {ldw_name}")
        p_ldw = pos[ldw_name]
        for m in prev_mms:
            if pos[m] >= p_ldw:
                raise RuntimeError(
                    f"PE order violation: {m} scheduled after {ldw_name}")
        for m in mm_names:
            if pos[m] <= p_ldw:
                raise RuntimeError(
                    f"PE order violation: {m} scheduled before {ldw_name}")
        prev_mms = mm_names


def _build_program(sched, S, repeat=1, loop_n=0):
    import contextlib
    waves = sched["waves"]
    acts = sched["acts"]
    acts_after = defaultdict(list)
    for rank, (w, b, colmap) in enumerate(acts):
        acts_after[w].append((b, rank))
    n_total_acts = len(acts)
    NA = n_total_acts

    nc = bass.Bass("TRN2", target_bir_lowering=False, debug=False,
                   num_devices=NCORES)
    x_d = nc.dram_tensor("xt", [128, 32 * BC], IN_DT, kind="ExternalInput").ap()
    w_d = nc.dram_tensor("wim", [128, S * 128], IN_DT, kind="ExternalInput").ap()
    b_d = nc.dram_tensor("bias", [128, NA], mybir.dt.float32,
                         kind="ExternalInput").ap()
    o_d = nc.dram_tensor("outT", [128, NA * BC], OUT_DT, kind="ExternalOutput").ap()

    NOSYNC = mybir.DependencyInfo.NO_SYNC_ONLY
    all_wave_names = []
    with tile.TileContext(nc) as tc:
        loop_cm = tc.For_i(0, loop_n, 1) if loop_n else contextlib.nullcontext()
        with tc.tile_pool(name="const", bufs=2) as cpool, \
             tc.tile_pool(name="work", bufs=4) as wpool, \
             tc.tile_pool(name="psum", bufs=1, space="PSUM") as ppool, \
             loop_cm:
            xt = cpool.tile([128, 32 * BC], IN_DT)
            wt = cpool.tile([128, S * 128], IN_DT)
            bt = cpool.tile([128, NA], mybir.dt.float32)
            nc.sync.dma_start(bt[:], b_d[:])
            # x: chunked DMA (8 x 1MB)
            xch = (32 * BC) // 8
            for k in range(8):
                nc.sync.dma_start(xt[:, k * xch:(k + 1) * xch],
                                  x_d[:, k * xch:(k + 1) * xch])
            # w: chunked DMA in wave order so early waves unblock early,
            # on the Activation HWDGE queue so x and w transfers use both
            # DMA queues in parallel
            wstep = -(-S // 8) * 128
            for k in range(8):
                lo = k * wstep
                hi = min(S * 128, lo + wstep)
                if lo >= hi:
                    continue
                (nc.scalar if DMA_SPLIT else nc.sync).dma_start(
                    wt[:, lo:hi], w_d[:, lo:hi])

            for rep in range(repeat):
                cur = {b: None for b in range(NBANKS)}
                gen = [0] * NBANKS
                prev_mms = []
                act_rank = 0
                otb = None
                for w, wave in enumerate(waves):
                    ldw = nc.tensor.ldweights(wt[:, 128 * w:128 * (w + 1)],
                                              tile_position=(0, 0))
                    ldw.ins.tile_size = (128, 128)
                    for m in prev_mms:
                        ldw.ins.add_dependency(m, NOSYNC)
                    mm_names = []
                    for (r, c, b, t, start, stop) in wave:
                        if cur[b] is None:
                            cur[b] = ppool.tile(
                                [128, BC], mybir.dt.float32, tag=f"acc{b}",
                                name=f"acc{b}_g{gen[b]}_p{rep}")
                            gen[b] += 1
                        mm = nc.tensor.matmul(
                            out=cur[b][32 * c:32 * c + 32, :],
                            lhsT=wt[32 * r:32 * r + 32,
                                    128 * w + 32 * c:128 * w + 32 * c + 32],
                            rhs=xt[32 * r:32 * r + 32, t * BC:(t + 1) * BC],
                            start=start, stop=stop,
                            tile_position=(32 * r, 32 * c),
                            skip_group_check=True,
                        )
                        mm.ins.ldweights = False
                        mm.ins.add_dependency(ldw.ins.name, NOSYNC)
                        mm_names.append(mm.ins.name)
                    all_wave_names.append((ldw.ins.name, mm_names))
                    prev_mms = mm_names
                    for (b, rank) in acts_after.get(w, []):
                        if act_rank % 2 == 0:
                            otb = wpool.tile([128, 2 * BC], OUT_DT, tag="otb")
                        ot = otb[:, (act_rank % 2) * BC:(act_rank % 2 + 1) * BC]
                        nc.scalar.activation(ot, cur[b][:],
                                             mybir.ActivationFunctionType.Relu,
                                             bias=bt[:, rank:rank + 1], scale=1.0)
                        if act_rank % 2 == 1:
                            # store issued from the ACT engine: same-engine
                            # dependency on the producer, no cross-engine sem
                            (nc.scalar if DMA_SPLIT else nc.sync).dma_start(
                                o_d[:, (act_rank - 1) * BC:(act_rank + 1) * BC],
                                otb[:])
                        elif act_rank == n_total_acts - 1:
                            (nc.scalar if DMA_SPLIT else nc.sync).dma_start(
                                o_d[:, act_rank * BC:(act_rank + 1) * BC],
                                otb[:, 0:BC])
                        act_rank += 1
                        cur[b] = None
                assert act_rank == n_total_acts
    _verify_pe_order(nc, all_wave_names)
    keep = {ldw for (ldw, mms) in all_wave_names}
    _strip_auto_ldws(nc, keep)
    _legalize_waits(nc)
    _strip_mm_sem_incs(nc)
    return nc


def _prep_inputs(x, w_blocks, bias, row_idx, col_idx, slot_of, dummy_slots, S):
    nnz = len(row_idx)
    acts = dummy_slots  # aux slot reused to carry the activation schedule
    # x^T images per core: block i at partitions 32*(i%4), free tile i//4.
    xb = x.astype(IN_NP).reshape(BATCH, 32, 4, 32)        # b, t, r, p
    xt_all = np.ascontiguousarray(xb.transpose(2, 3, 1, 0))  # r, p, t, b
    xt_all = xt_all.reshape(128, 32, BATCH)
    xts = [np.ascontiguousarray(xt_all[:, :, c * BC:(c + 1) * BC]
                                ).reshape(128, 32 * BC) for c in range(NCORES)]
    # w image [128, S*128]: wave w, tile (r, c) at [32r:, 128w+32c:]
    wim = np.zeros((128, S * 128), dtype=IN_NP)
    wb = w_blocks.astype(IN_NP)
    for n in range(nnz):
        w, r, c = slot_of[n]
        wim[32 * r:32 * r + 32, 128 * w + 32 * c:128 * w + 32 * c + 32] = wb[n]
    # bias image [128, n_acts]: ACT rank g hosts cols colmap[c] at strips c
    bias = bias.astype(np.float32)
    bim = np.zeros((128, len(acts)), dtype=np.float32)
    for rank, (w, b, colmap) in enumerate(acts):
        for c, j in colmap.items():
            bim[32 * c:32 * c + 32, rank] = bias[32 * j:32 * j + 32]
    return xts, wim, bim


def kernel(x, w_blocks, bias, row_idx, col_idx):
    x = np.asarray(x)
    w_blocks = np.asarray(w_blocks)
    bias = np.asarray(bias)
    row_idx = np.asarray(row_idx)
    col_idx = np.asarray(col_idx)
    key = (row_idx.tobytes(), col_idx.tobytes())
    if key not in _CACHE:
        sched, S, slot_of, dummy_slots = _build_schedule(row_idx, col_idx)
        nc = _build_program(sched, S)
        _CACHE[key] = (nc, S, (sched, slot_of, dummy_slots))
    nc, S, aux = _CACHE[key]
    sched, slot_of, dummy_slots = aux
    acts = sched["acts"]

    xts, wim, bim = _prep_inputs(x, w_blocks, bias, row_idx, col_idx,
                                 slot_of, acts, S)
    in_maps = [{"xt": xts[c], "wim": wim, "bias": bim} for c in range(NCORES)]
    res = run_bass_kernel_spmd(nc, in_maps, list(range(NCORES)))
    global LAST_RESULTS
    LAST_RESULTS = res

    out = np.empty((BATCH, NB * BS), dtype=np.float32)
    for c in range(NCORES):
        arr = res.results[c]["outT"].reshape(128, len(acts), BC)
        oc = out[c * BC:(c + 1) * BC, :]
        for rank, (w, b, colmap) in enumerate(acts):
            for st, j in colmap.items():
                oc[:, 32 * j:32 * j + 32] = (
                    arr[32 * st:32 * st + 32, rank, :].T.astype(np.float32))
    return out
